# revision 7
# baseline (speedup 1.0000x reference)
"""ChebConv (K=8) on 8 Trainium2 NeuronCores.

Sharding: destination-node sharding. Core m owns rows [2500m, 2500(m+1)),
padded to 2560 rows (20 chunks of 128). Per hop:
  - dma_gather fp16 messages x[col[e]] from the (all-gathered) node table
  - one-hot matmul scatter (lap folded into the one-hot) -> PSUM fp32
  - DVE: Tx_k = psum(2*lap) - Tx_{k-2} (fp16), written to a DRAM shard
  - AllGather shards -> next hop's gather table
  - dense: out^T += W_k^T-chunks @ Tx_k^T (DMA-transpose loads of own shard)
Out accumulates transposed fp32 in SBUF; bias added at the end.
The 2*lap one-hot is built on device once (hop 1) and stays SBUF-resident.
"""

import os
import sys
import numpy as np

sys.path.insert(0, "/opt/trn_rl_repo")

K_HOPS = int(os.environ.get("K_HOPS", "8"))
REPEAT = int(os.environ.get("REPEAT", "1"))

import concourse.bass as bass
import concourse.mybir as mybir
import concourse.tile as tile
import concourse.bacc as bacc
from concourse import library_config
from concourse.bass_utils import run_bass_kernel_spmd

N = 20000
E = 150000
C = 512
KW = 8
NCORES = 8
P = 128
RPC = N // NCORES          # real rows per core: 2500
RCH = 20                   # row chunks per core
RP = RCH * P               # padded rows per core: 2560
TBL = NCORES * RP          # padded global table rows: 20480
NG = RP // 512             # node groups of 512 for dense: 5
KC = C // P                # k chunks: 4
OC = C // P                # out-channel chunks: 4

_BUILD_CACHE = {}


def _host_prep(x, edge_index, weight, bias):
    row = np.asarray(edge_index[0], dtype=np.int64)
    col = np.asarray(edge_index[1], dtype=np.int64)
    x = np.asarray(x, dtype=np.float32)
    weight = np.asarray(weight, dtype=np.float32)
    bias = np.asarray(bias, dtype=np.float32)

    deg = np.bincount(row, minlength=N).astype(np.float32)
    dinv = np.where(deg > 0, 1.0 / np.sqrt(np.maximum(deg, 1e-30)), 0.0).astype(
        np.float32
    )
    lap = (-dinv[row] * dinv[col]).astype(np.float32)

    # global padded index of node n in the 20480-row table
    gcol = (col // RPC) * RP + (col % RPC)

    core = row // RPC
    lrow = row % RPC
    chunk = lrow // P
    lr = lrow % P

    order = np.lexsort((chunk, core))
    core_s, chunk_s = core[order], chunk[order]
    lr_s, gcol_s, lap_s = lr[order], gcol[order], lap[order]

    counts = np.zeros((NCORES, RCH), dtype=np.int64)
    np.add.at(counts, (core_s, chunk_s), 1)
    m_ec = int(np.ceil(counts.max() / P))
    epc = m_ec * P

    idx_pad = np.zeros((NCORES, RCH, epc), dtype=np.int16)
    oh1 = np.zeros((NCORES, RCH, epc, P), dtype=np.float16)

    starts = np.zeros((NCORES, RCH), dtype=np.int64)
    flat_counts = counts.reshape(-1)
    np.cumsum(flat_counts[:-1], out=starts.reshape(-1)[1:])
    pos_in_chunk = np.arange(len(order)) - starts[core_s, chunk_s]

    idx_pad[core_s, chunk_s, pos_in_chunk] = gcol_s.astype(np.int16)
    oh1[core_s, chunk_s, pos_in_chunk, lr_s] = lap_s.astype(np.float16)

    # dma_gather idx layout per row chunk: idx i at partition i%16, col i//16,
    # replicated 8x down the 128 partitions -> [RCH, 128, epc//16] per core
    blk = idx_pad.reshape(NCORES, RCH, epc // 16, 16).transpose(0, 1, 3, 2)
    gidx = np.tile(blk, (1, 1, 8, 1))

    xpad = np.zeros((TBL, C), dtype=np.float16)
    xpad.reshape(NCORES, RP, C)[:, :RPC] = x.reshape(NCORES, RPC, C)
    xloc = xpad.reshape(NCORES, RP, C)

    w16 = weight.astype(np.float16)
    bias_r = bias.reshape(OC, P).T.astype(np.float32)
    bias_r = np.ascontiguousarray(bias_r)

    in_maps = []
    for m in range(NCORES):
        in_maps.append(
            {
                "xloc": np.ascontiguousarray(xloc[m]),
                "gidx": np.ascontiguousarray(gidx[m].reshape(RCH * P, epc // 16)),
                "oh1": np.ascontiguousarray(oh1[m]),
                "w": w16,
                "biasr": bias_r,
            }
        )
    return in_maps, m_ec


def _build(m_ec):
    epc = m_ec * P
    nc = bacc.Bacc("TRN2", target_bir_lowering=False, debug=False, num_devices=NCORES)
    dt = mybir.dt

    xloc = nc.dram_tensor("xloc", [RP, C], dt.float16, kind="ExternalInput")
    gidx = nc.dram_tensor("gidx", [RCH * P, epc // 16], dt.int16, kind="ExternalInput")
    oh1 = nc.dram_tensor("oh1", [RCH, epc, P], dt.float16, kind="ExternalInput")
    w = nc.dram_tensor("w", [KW, C, C], dt.float16, kind="ExternalInput")
    biasr = nc.dram_tensor("biasr", [P, OC], dt.float32, kind="ExternalInput")
    out = nc.dram_tensor("out", [C, RP], dt.float32, kind="ExternalOutput")

    with tile.TileContext(nc) as tc:
        nc.gpsimd.load_library(library_config.mlp)
        with (
            tc.tile_pool(name="const", bufs=1) as constp,
            tc.tile_pool(name="work", bufs=3) as workp,
            tc.tile_pool(name="txt", bufs=8) as txtp,
            tc.tile_pool(name="acc", bufs=1) as accp,
            tc.tile_pool(name="pss", bufs=2, space="PSUM") as pss,
            tc.tile_pool(name="psd", bufs=5, space="PSUM") as psd,
            tc.tile_pool(name="dram", bufs=3, space="DRAM") as dramp,
        ):
            # ---- constants loaded once ----
            idx_t = constp.tile([P, RCH, epc // 16], dt.int16, name="idx_t")
            nc.sync.dma_start(
                idx_t[:], gidx.ap().rearrange("(ch p) s -> p ch s", p=P)
            )
            w_t = constp.tile([P, KW, KC, C], dt.float16, name="w_t")
            nc.sync.dma_start(
                w_t[:], w.ap().rearrange("k (kc p) o -> p k kc o", p=P)
            )
            bias_t = constp.tile([P, OC], dt.float32, name="bias_t")
            nc.sync.dma_start(bias_t[:], biasr[:])
            # SBUF-resident 2*lap one-hot, filled during hop 1
            oh2_t = constp.tile([P, RCH, m_ec, P], dt.float16, name="oh2_t")

            outacc = [
                [
                    accp.tile([P, 512], dt.float32, name=f"oacc_{o}_{g}", tag=f"oa{o}{g}")
                    for g in range(NG)
                ]
                for o in range(OC)
            ]

            for rep in range(REPEAT):
                shards = {}
                tables = {}

                def dense(k, rep=rep, shards=shards):
                    for g in range(NG):
                        txt = []
                        for kc in range(KC):
                            t = txtp.tile(
                                [P, 512], dt.float16,
                                name=f"txt_{rep}_{k}_{g}_{kc}", tag="txt",
                            )
                            src = xloc if k == 0 else shards[k]
                            nc.sync.dma_start_transpose(
                                t[:],
                                src[g * 512 : (g + 1) * 512, kc * P : (kc + 1) * P],
                            )
                            txt.append(t)
                        for o in range(OC):
                            ps = psd.tile(
                                [P, 512], dt.float32,
                                name=f"psd_{rep}_{k}_{g}_{o}", tag="psd",
                            )
                            for kc in range(KC):
                                nc.tensor.matmul(
                                    ps[:],
                                    w_t[:, k, kc, o * P : (o + 1) * P],
                                    txt[kc][:],
                                    start=(kc == 0),
                                    stop=(kc == KC - 1),
                                )
                            if k == 0:
                                nc.vector.tensor_copy(outacc[o][g][:], ps[:])
                            else:
                                nc.vector.tensor_tensor(
                                    outacc[o][g][:], outacc[o][g][:], ps[:],
                                    op=mybir.AluOpType.add,
                                )
                            if k == KW - 1:
                                nc.vector.tensor_scalar_add(
                                    outacc[o][g][:], outacc[o][g][:],
                                    bias_t[:, o : o + 1],
                                )
                                nc.sync.dma_start(
                                    out[o * P : (o + 1) * P, g * 512 : (g + 1) * 512],
                                    outacc[o][g][:],
                                )

                # gather table gen 0 = all-gathered x (bounce: collectives
                # cannot use kernel I/O tensors directly)
                xb = dramp.tile([RP, C], dt.float16, name=f"xb_{rep}", tag="shard")
                nc.sync.dma_start(xb[:], xloc[:])
                tab0 = dramp.tile([TBL, C], dt.float16, name=f"tab0_{rep}", tag="table")
                tables[0] = tab0
                nc.gpsimd.collective_compute(
                    "AllGather",
                    mybir.AluOpType.bypass,
                    replica_groups=[list(range(NCORES))],
                    ins=[xb.opt()],
                    outs=[tab0.opt()],
                )

                dense(0)

                for k in range(1, K_HOPS):
                    shard = dramp.tile(
                        [RP, C], dt.float16, name=f"shard_{rep}_{k}", tag="shard"
                    )
                    shards[k] = shard
                    for ch in range(RCH):
                        msgs = workp.tile(
                            [P, m_ec, C], dt.float16,
                            name=f"m_{rep}_{k}_{ch}", tag="msgs",
                        )
                        for j0 in range(0, m_ec, 8):
                            j1 = min(j0 + 8, m_ec)
                            nidx = (j1 - j0) * P
                            nc.gpsimd.dma_gather(
                                msgs[:, j0:j1, :],
                                tables[k - 1][:],
                                idx_t[:, ch, j0 * 8 : j1 * 8],
                                nidx,
                                nidx,
                                C,
                            )
                        if k == 1:
                            oht = workp.tile(
                                [P, m_ec, P], dt.float16,
                                name=f"o_{rep}_{ch}", tag="oht",
                            )
                            nc.sync.dma_start(
                                oht[:], oh1.ap()[ch].rearrange("(a b) c -> b a c", b=P)
                            )
                            # resident 2x one-hot for hops 2..7
                            nc.vector.tensor_scalar_mul(
                                oh2_t[:, ch], oht[:], 2.0
                            )
                        else:
                            oht = None
                        ps = pss.tile(
                            [P, C], dt.float32, name=f"pss_{rep}_{k}_{ch}", tag="pss"
                        )
                        lhs = oht if k == 1 else None
                        for j in range(m_ec):
                            nc.tensor.matmul(
                                ps[:],
                                (lhs[:, j, :] if k == 1 else oh2_t[:, ch, j, :]),
                                msgs[:, j, :],
                                start=(j == 0),
                                stop=(j == m_ec - 1),
                            )
                        st = workp.tile(
                            [P, C], dt.float16, name=f"st_{rep}_{k}_{ch}", tag="stage"
                        )
                        if k == 1:
                            nc.vector.tensor_copy(st[:], ps[:])
                        else:
                            sub_src = xloc if k == 2 else shards[k - 2]
                            sb = workp.tile(
                                [P, C], dt.float16,
                                name=f"sb_{rep}_{k}_{ch}", tag="subt",
                            )
                            nc.sync.dma_start(sb[:], sub_src[ch * P : (ch + 1) * P, :])
                            nc.vector.tensor_tensor(
                                st[:], ps[:], sb[:], op=mybir.AluOpType.subtract
                            )
                        nc.sync.dma_start(shard[ch * P : (ch + 1) * P, :], st[:])
                    if k < K_HOPS - 1:
                        table = dramp.tile(
                            [TBL, C], dt.float16, name=f"tab_{rep}_{k}", tag="table"
                        )
                        tables[k] = table
                        nc.gpsimd.collective_compute(
                            "AllGather",
                            mybir.AluOpType.bypass,
                            replica_groups=[list(range(NCORES))],
                            ins=[shard.opt()],
                            outs=[table.opt()],
                        )
                    dense(k)

    nc.compile()
    return nc


def kernel(x, edge_index, weight, bias):
    in_maps, m_ec = _host_prep(x, edge_index, weight, bias)
    key = (m_ec, K_HOPS, REPEAT)
    if key not in _BUILD_CACHE:
        _BUILD_CACHE[key] = _build(m_ec)
    nc = _BUILD_CACHE[key]
    r = run_bass_kernel_spmd(nc, in_maps, core_ids=list(range(NCORES)))
    outT = np.concatenate(
        [r.results[m]["out"][:, :RPC] for m in range(NCORES)], axis=1
    )  # [C, N]
    return np.ascontiguousarray(outT.T).astype(np.float32)
